# revision 48
# baseline (speedup 1.0000x reference)
"""Trainium2 Bass kernel for nn_EngramModule: single-query top-k memory attention
with gated residual + LayerNorm, data-parallel across 8 NeuronCores.

Contract: kernel(**inputs) takes the FULL unsharded inputs and returns the FULL
(8192, 1024) float32 output.

Per-core design (1024 batch rows, 8 row-tiles of 128), one software-pipelined
loop emitting [B(t), A(t+1), C(t-1)] per iteration so TensorE never re-ramps
its pstate across phase boundaries:
  A:  Q = h @ Wq (bf16); one stage ahead of B(t)
  B:  per (tile, k): Kp AND Vp both fp8-DoubleRow (2x) projections on TensorE;
      kp copied to bf16 SBUF on ScalarE so the score mul+reduce run in DVE 2x
      packed mode; online softmax (no max-subtraction; logits ~N(0,1));
      attn-weighted V + the k-acc chain in bf16 on DVE (full width, 2x adds)
  C:  memory_out = attnout @ Wo (bf16); gate = [h|mo] @ Wg, both halves
      fp8-DoubleRow; transposes in bf16 (1 cyc/row) through bitcast views
      of a shared fp32 PSUM tag; sigmoid via 0.5*tanh(x/2)+0.5 (one ACT
      table set with exp); residual + gate epilogue in bf16 (h streamed as
      bf16), LayerNorm stats/finalize fp32 with VectorE-only Newton rsqrt,
      epilogue in 512-wide halves so the last tile's tail pipelines.

Startup: weight loads ride a handful of BIG DMA descriptors (each ~600ns of
trigger time on the issuing engine; the transfer itself auto-fans across all
16 DMA engines by partition), split across the scalar+vector queues so
trigger issue parallelizes; eyeb rides the head of the sync queue and a block
of dependency-free transposes warms the PE (HAM un-throttles after ~3.4us of
sustained activity) while weights stream.

Precision (validated against a host emulation that matches HW to ~1%):
memory_keys/Wk, memory_values/Wv and both Wg halves in fp8e4m3 (weights
pre-scaled x16, undone downstream), Q and Wo in bf16, score accumulate fp32,
attention weights/products bf16, residual in bf16, LayerNorm stats fp32.

PSUM is exactly 8 banks: Kp/Vp cycle one shared [128,1024]f32 tag; Q-proj,
ao-transpose, mo, mo-transpose and gate cycle the other (the last tile's gate
borrows the Kp/Vp tag so it can start during the final softmax drain).
"""

import os
import sys

import numpy as np

for _p in ("/opt/trn_rl_repo", "/root/.axon_site/_ro/trn_rl_repo"):
    if os.path.isdir(_p) and _p not in sys.path:
        sys.path.insert(0, _p)

from contextlib import ExitStack

import concourse.bacc as bacc
import concourse.mybir as mybir
import concourse.tile as tile
from concourse.bass_utils import run_bass_kernel_spmd

F32 = mybir.dt.float32
BF16 = mybir.dt.bfloat16
F8 = mybir.dt.float8e4
I32 = mybir.dt.int32
AX = mybir.AxisListType
OP = mybir.AluOpType
AF = mybir.ActivationFunctionType
DR = mybir.MatmulPerfMode.DoubleRow

# fp8 weights are pre-scaled by WS on host so their mass sits in e4m3's
# normal range; the 1/WS is folded into downstream constants.
WS = 16.0

N_CORES = 8
B = 8192
HID = 1024
NH = 16
DH = 64
TOPK = 8
LN_EPS = 1e-5

BC = B // N_CORES          # rows per core = 1024
NT = BC // 128             # row-tiles per core = 8
NIC = HID // 128           # 128-row contraction chunks = 8
NJH = HID // 512           # 512-wide output halves = 2
SCALE = DH ** -0.5
RSQRT_MAGIC = 0x5F3759DF

# Set by test.py to collect a profile; grading path leaves this off.
TRACE = False

_CACHE = {}


def _build(nt=NT):
    nc = bacc.Bacc("TRN2", target_bir_lowering=False, debug=False,
                   num_devices=N_CORES)

    # ---- DRAM parameters (per-core shard, host-prepped layouts) ----
    hb_d = nc.declare_dram_parameter("hB", [nt, 128, HID], BF16, isOutput=False)
    # K slots 0..TOPK-2 are fp8; the last slot stays bf16 to keep the joint
    # quantization noise of the score path under the accuracy budget
    mkT_d = nc.declare_dram_parameter("mkT", [nt, TOPK - 1, 128, NIC, 128], F8, isOutput=False)
    mkTb_d = nc.declare_dram_parameter("mkTb", [nt, 128, NIC, 128], BF16, isOutput=False)
    mvT_d = nc.declare_dram_parameter("mvT", [nt, TOPK, 128, NIC, 128], F8, isOutput=False)
    hTb_d = nc.declare_dram_parameter("hTb", [nt, 128, NIC, 128], BF16, isOutput=False)
    hT8_d = nc.declare_dram_parameter("hT8", [nt, 128, NIC, 128], F8, isOutput=False)
    wq_d = nc.declare_dram_parameter("Wq", [128, NIC, HID], BF16, isOutput=False)
    wk_d = nc.declare_dram_parameter("Wk", [128, NIC, HID], F8, isOutput=False)
    wkb_d = nc.declare_dram_parameter("WkB", [128, NIC, HID], BF16, isOutput=False)
    wv_d = nc.declare_dram_parameter("Wv", [128, NIC, HID], F8, isOutput=False)
    wo_d = nc.declare_dram_parameter("Wo", [128, NIC, HID], BF16, isOutput=False)
    # gate = sigmoid([h|mo] @ Wg + bg) is computed as h @ Wg1 + attnout @
    # (Wo @ Wg2): the fused weight is built on host, so the gate's mo-half
    # reuses the already-transposed attnout and the mo transpose disappears
    wg_d = nc.declare_dram_parameter("Wg", [128, NIC, HID], F8, isOutput=False)
    wf_d = nc.declare_dram_parameter("Wf", [128, NIC, HID], F8, isOutput=False)
    bgb_d = nc.declare_dram_parameter("bgB", [128, HID], F32, isOutput=False)
    eyeb_d = nc.declare_dram_parameter("eyeB", [128, 128], BF16, isOutput=False)
    lng_d = nc.declare_dram_parameter("lngB", [128, HID], F32, isOutput=False)
    lnb_d = nc.declare_dram_parameter("lnbB", [128, HID], F32, isOutput=False)
    out_d = nc.declare_dram_parameter("out", [nt, 128, HID], F32, isOutput=True)

    def load_w(tile_sb, dram, nchunk, step, q):
        # bulk weights in a handful of big descriptors: each dma_start costs
        # the issuing engine ~600ns of trigger time, so fewer+bigger wins;
        # the transfer itself fans across all 16 DMA engines by partition.
        # step is chosen so the first dependent matmuls only wait for their
        # own chunk while keeping per-partition lines >= 4KB.
        for ic in range(0, nchunk, step):
            q.dma_start(tile_sb[:, ic:ic + step, :],
                        dram.ap()[:, ic:ic + step, :])

    with ExitStack() as octx:
        tc = octx.enter_context(tile.TileContext(nc))

        # Wo/Wg are loaded during B(0) (gpsimd queue) and consumed in C
        pWO_cm = tc.tile_pool(name="pWO", bufs=1); pWO = pWO_cm.__enter__()
        wo_sb = pWO.tile([128, NIC, HID], BF16, tag="wo")
        wg_sb = pWO.tile([128, NIC, HID], F8, tag="wg")
        wf_sb = pWO.tile([128, NIC, HID], F8, tag="wf")

        # attnout stays SBUF-resident from B into C (no DRAM spill); bf16 so
        # the PE transpose runs 1 cycle/row instead of fp32's 2 (it is
        # quantized to bf16 before the Wo matmul anyway)
        pAO_cm = tc.tile_pool(name="pAO", bufs=1); pAO = pAO_cm.__enter__()
        ao_all = pAO.tile([128, nt, HID], BF16, tag="ao_all")

        # q tiles are streamed: produced one pipeline stage ahead, consumed
        # by the next tile's B stage (bf16: feeds the score mult only)
        pAB_cm = tc.tile_pool(name="pAB", bufs=3); pAB = pAB_cm.__enter__()

        # eyeb rides the HEAD of the sync queue: the PE warm-up transposes
        # (emitted before A(0)) depend only on it
        eyep_cm = tc.tile_pool(name="eye", bufs=1); eyep = eyep_cm.__enter__()
        eyeb_sb = eyep.tile([128, 128], BF16, tag="eyeb")
        nc.sync.dma_start(eyeb_sb[:], eyeb_d.ap())

        # Wq/Wk/Wv stream in up front: wq on the scalar queue, wk/wv on the
        # gpsimd queue so the two engines issue triggers in parallel
        pWKV_cm = tc.tile_pool(name="wkv", bufs=1); wkv = pWKV_cm.__enter__()
        wq_sb = wkv.tile([128, NIC, HID], BF16, tag="wq")
        wk_sb = wkv.tile([128, NIC, HID], F8, tag="wk")
        wkb_sb = wkv.tile([128, NIC, HID], BF16, tag="wkb")
        wv_sb = wkv.tile([128, NIC, HID], F8, tag="wv")

        # wq chunks alternate between the two trigger queues so the per-queue
        # transfer serialization doesn't gate the Q projection's chunk stream
        for ic in range(NIC):
            q = nc.scalar if ic % 2 == 0 else nc.gpsimd
            q.dma_start(wq_sb[:, ic:ic + 1, :], wq_d.ap()[:, ic:ic + 1, :])
        load_w(wk_sb, wk_d, NIC, step=4, q=nc.scalar)
        load_w(wv_sb, wv_d, NIC, step=4, q=nc.gpsimd)
        # the bf16 K weights are only needed at each tile's LAST k-slot, so
        # they trail the startup-critical streams on the scalar queue
        load_w(wkb_sb, wkb_d, NIC, step=4, q=nc.scalar)

        # hidden-state tiles for the Q projection are streamed two pipeline
        # stages ahead rather than fully resident
        hp_cm = tc.tile_pool(name="hT_a", bufs=3); hp = hp_cm.__enter__()
        hstream = {}

        def load_hT(t):
            # one descriptor per tensor: a dma_start costs the issuing engine
            # ~580ns of trigger time, and the transfer itself already fans
            # out across all 16 DMA engines by partition line
            hT_t = hp.tile([128, NIC, 128], BF16, tag="hT")
            nc.sync.dma_start(hT_t[:], hTb_d.ap()[t])
            hstream[t] = hT_t

        load_hT(0)
        load_hT(1)

        # ===== merged B|C: per-tile attention + output, software-pipelined =====
        # Emission order B(0), B(1), C(0), B(2), C(1), ... keeps TensorE
        # continuously warm (no phase boundary, no pstate re-ramp) and lets
        # C(t)'s Vector/Scalar epilogue drain while B(t+1) runs matmuls.
        # PSUM budget is exactly 8 banks: kp/vp share one cycled tag (4) and
        # all C-phase psum users (aoT, mo, moT, gate) share another (4).
        with ExitStack() as bctx:
            mp = bctx.enter_context(tc.tile_pool(name="mkv", bufs=6))
            # PSUM: 8 banks = kv ring 3x[128,1024] (6) + big ring 1x (2).
            # The kv ring carries kp/vp/gate: distance-3 reuse gives every
            # bank ~2 full PE slots of consumer slack. The big ring carries
            # q/aoT/mo, which are data-serial anyway.
            kvps = bctx.enter_context(tc.tile_pool(name="kv_ps", bufs=3, space="PSUM"))
            bigps = bctx.enter_context(tc.tile_pool(name="big_ps", bufs=1, space="PSUM"))
            sp = bctx.enter_context(tc.tile_pool(name="scr", bufs=2))
            accp = bctx.enter_context(tc.tile_pool(name="acc", bufs=2))
            ep = bctx.enter_context(tc.tile_pool(name="e", bufs=2))
            cstr = bctx.enter_context(tc.tile_pool(name="c_str", bufs=2))
            csb = bctx.enter_context(tc.tile_pool(name="c_sb", bufs=2))
            stp = bctx.enter_context(tc.tile_pool(name="stats", bufs=2))

            def warm_pe(n, pool):
                # dependency-free transposes hold the PE at full pstate while
                # other engines catch up (HAM re-throttles after ~3.4us idle,
                # and a cold PE runs matmuls at half clock)
                scr_ps = pool.tile([128, HID], F32,
                                   tag="kv" if pool is kvps else "big")
                scr_v = scr_ps[:].bitcast(BF16)
                for d in range(n):
                    nc.tensor.transpose(
                        scr_v[:, (d % 16) * 128:(d % 16 + 1) * 128],
                        eyeb_sb[:], eyeb_sb[:])

            # startup: warm the PE during the initial weight/activation DMA
            warm_pe(48, kvps)

            def load_kv(t, k):
                if k == TOPK - 1:
                    a = mp.tile([128, NIC, 128], BF16, tag="mkTb")
                    nc.sync.dma_start(a[:], mkTb_d.ap()[t])
                else:
                    a = mp.tile([128, NIC, 128], F8, tag="mkT")
                    nc.sync.dma_start(a[:], mkT_d.ap()[t, k])
                b_ = mp.tile([128, NIC, 128], F8, tag="mvT")
                nc.sync.dma_start(b_[:], mvT_d.ap()[t, k])
                return (a, b_)

            preload = {(0, k): load_kv(0, k) for k in range(2)}

            cstream = {}
            qstream = {}
            e_ref = {}

            def emit_A(t):
                # Q projection for tile t, one pipeline stage ahead of its
                # B(t); shares the "big" psum tag with the C-phase users.
                hT_t = hstream.pop(t)
                q_ps = bigps.tile([128, HID], F32, tag="big")
                for ic in range(NIC):
                    for jh in range(NJH):
                        nc.tensor.matmul(
                            q_ps[:, jh * 512:(jh + 1) * 512],
                            hT_t[:, ic, :],
                            wq_sb[:, ic, jh * 512:(jh + 1) * 512],
                            start=(ic == 0), stop=(ic == NIC - 1),
                        )
                q_sb = pAB.tile([128, HID], BF16, tag="q")
                nc.scalar.copy(q_sb[:], q_ps[:])
                qstream[t] = q_sb

            def emit_weighted(t, k, vp_ps, e_all, acc):
                # weighted V accumulate: acc += e[:,k,h] (bcast over d) * Vp;
                # runs one k-slot deferred so vp_ps is always ready when the
                # DVE queue reaches it (no head-of-line stall)
                e_bc = e_all[:, k, :].unsqueeze(2).broadcast_to([128, NH, DH])
                dst = acc if k == 0 else sp.tile([128, HID], BF16, tag="pv")
                nc.vector.tensor_tensor(
                    dst[:].rearrange("p (h d) -> p h d", h=NH),
                    vp_ps[:].rearrange("p (h d) -> p h d", h=NH),
                    e_bc, op=OP.mult)
                if k > 0:
                    # the serial acc chain split across DVE/GpSimd halves
                    nc.gpsimd.tensor_add(acc[:, :512], acc[:, :512],
                                         dst[:, :512])
                    nc.vector.tensor_add(acc[:, 512:], acc[:, 512:],
                                         dst[:, 512:])

            def emit_vp(mvT):
                vp_ps = kvps.tile([128, HID], F32, tag="kv")
                for j in range(NIC // 2):
                    for jh in range(NJH):
                        nc.tensor.matmul(
                            vp_ps[:, jh * 512:(jh + 1) * 512],
                            mvT[:, 2 * j:2 * j + 2, :],
                            wv_sb[:, 2 * j:2 * j + 2, jh * 512:(jh + 1) * 512],
                            start=(j == 0), stop=(j == NIC // 2 - 1),
                            perf_mode=DR,
                        )
                return vp_ps

            def emit_cstream(t):
                # tile t's C-phase streams, prefetched at least half a tile
                # ahead of their C(t) consumption
                hT_sb = cstr.tile([128, NIC, 128], F8, tag="hT_c")
                nc.sync.dma_start(hT_sb[:], hT8_d.ap()[t])
                h_sb = cstr.tile([128, HID], BF16, tag="h_c")
                nc.sync.dma_start(h_sb[:], hb_d.ap()[t])
                cstream[t] = (hT_sb, h_sb)

            def emit_B(t):
                if t > 0:
                    emit_cstream(t)

                acc = accp.tile([128, HID], BF16, tag="acc")
                e_all = ep.tile([128, TOPK, NH], F32, tag="e_all")
                e_ref[t] = e_all
                q_t = qstream.pop(t)[:]
                prev = None   # (k, mvT) whose V projection is not yet emitted
                for k in range(TOPK):
                    if (t, k) in preload:
                        mkT, mvT = preload[(t, k)]
                    else:
                        mkT, mvT = load_kv(t, k)

                    # fp8 DoubleRow: each matmul contracts TWO 128-chunks
                    # (stationary [128,2,128] data rows, moving [128,2,512]
                    # weights); Wk/Wv are pre-scaled by WS (undone in the
                    # exp scale / in Wo respectively). The last k-slot runs
                    # bf16 for accuracy. Slot k's V projection is emitted
                    # AFTER slot k+1's K projection: the weighted-V consumer
                    # (the longest DVE stage) then has a full extra slot to
                    # free vp's PSUM bank before the next V matmuls need it.
                    kp_ps = kvps.tile([128, HID], F32, tag="kv")
                    if k == TOPK - 1:
                        for ic in range(NIC):
                            for jh in range(NJH):
                                nc.tensor.matmul(
                                    kp_ps[:, jh * 512:(jh + 1) * 512],
                                    mkT[:, ic, :],
                                    wkb_sb[:, ic, jh * 512:(jh + 1) * 512],
                                    start=(ic == 0), stop=(ic == NIC - 1),
                                )
                    else:
                        for j in range(NIC // 2):
                            for jh in range(NJH):
                                nc.tensor.matmul(
                                    kp_ps[:, jh * 512:(jh + 1) * 512],
                                    mkT[:, 2 * j:2 * j + 2, :],
                                    wk_sb[:, 2 * j:2 * j + 2, jh * 512:(jh + 1) * 512],
                                    start=(j == 0), stop=(j == NIC // 2 - 1),
                                    perf_mode=DR,
                                )
                    if prev is not None:
                        vp_prev = emit_vp(prev[1])

                    # kp -> bf16 SBUF on ScalarE: frees kp's PSUM bank after
                    # a short copy (instead of after the DVE mul) and lets
                    # the score mul+reduce run in DVE 2x packed mode
                    kp_sb = sp.tile([128, HID], BF16, tag="kpb")
                    nc.scalar.copy(kp_sb[:], kp_ps[:])
                    p_scr = sp.tile([128, HID], BF16, tag="p")
                    nc.vector.tensor_mul(p_scr[:], q_t, kp_sb[:])
                    s_k = ep.tile([128, NH], F32, tag="s_k")
                    nc.vector.reduce_sum(
                        s_k[:], p_scr[:].rearrange("p (h d) -> p h d", h=NH), axis=AX.X)
                    # e = exp(scores * DH**-0.5 / WS); logits ~N(0,1) so no
                    # max-sub; 1/WS undoes the Wk pre-scale (fp8 slots only)
                    nc.scalar.activation(e_all[:, k, :], s_k[:], AF.Exp,
                                         scale=SCALE if k == TOPK - 1 else SCALE / WS)

                    if prev is not None:
                        emit_weighted(t, prev[0], vp_prev, e_all, acc)
                    prev = (k, mvT)
                vp_last = emit_vp(prev[1])
                emit_weighted(t, prev[0], vp_last, e_all, acc)

                # normalize: attnout = acc * (1/sum_k e)
                den = ep.tile([128, NH], F32, tag="den")
                nc.vector.reduce_sum(
                    den[:], e_all[:].rearrange("p k h -> p h k"), axis=AX.X)
                rden = ep.tile([128, NH], F32, tag="rden")
                nc.vector.reciprocal(rden[:], den[:])
                rden_bc = rden[:].unsqueeze(2).broadcast_to([128, NH, DH])
                nc.vector.tensor_tensor(
                    ao_all[:, t, :].rearrange("p (h d) -> p h d", h=NH),
                    acc[:].rearrange("p (h d) -> p h d", h=NH),
                    rden_bc, op=OP.mult)

            def emit_gate_h(hT_sb, g_ps):
                # gate h-part: depends only on hT8(t) + Wg, so it can run
                # while the softmax/normalize for tile t still drains
                for j in range(NIC // 2):
                    for jh in range(NJH):
                        sl = slice(jh * 512, (jh + 1) * 512)
                        nc.tensor.matmul(
                            g_ps[:, sl], hT_sb[:, 2 * j:2 * j + 2, :],
                            wg_sb[:, 2 * j:2 * j + 2, sl],
                            start=(j == 0), stop=False, perf_mode=DR)

            def emit_C(t, g_ps=None):
                hT_sb, h_sb = cstream.pop(t)

                # transpose attnout (bf16, 1 cyc/row) into a bf16 view of a
                # shared fp32 psum tag; copy out in halves so the first mo
                # matmuls start while the second half still copies
                aoT_ps = bigps.tile([128, HID], F32, tag="big")
                aoT_v = aoT_ps[:].bitcast(BF16)
                for ic in range(NIC):
                    nc.tensor.transpose(
                        aoT_v[:, ic * 128:(ic + 1) * 128],
                        ao_all[:, t, ic * 128:(ic + 1) * 128],
                        eyeb_sb[:])
                atT_sb = csb.tile([128, NIC, 128], BF16, tag="atT")
                atT8_sb = csb.tile([128, NIC, 128], F8, tag="atT8")
                half = NIC // 2 * 128
                last = g_ps is not None
                if last:
                    # last tile: the gate (fused mo-part) paces the exposed
                    # tail, so its fp8 stationary copies go first
                    for c in range(2):
                        nc.scalar.copy(
                            atT8_sb[:, c * NIC // 2:(c + 1) * NIC // 2],
                            aoT_v[:, c * half:(c + 1) * half].rearrange(
                                "p (a b) -> p a b", a=NIC // 2))
                for c in range(2):
                    nc.scalar.copy(
                        atT_sb[:, c * NIC // 2:(c + 1) * NIC // 2],
                        aoT_v[:, c * half:(c + 1) * half].rearrange(
                            "p (a b) -> p a b", a=NIC // 2))
                if not last:
                    for c in range(2):
                        nc.scalar.copy(
                            atT8_sb[:, c * NIC // 2:(c + 1) * NIC // 2],
                            aoT_v[:, c * half:(c + 1) * half].rearrange(
                                "p (a b) -> p a b", a=NIC // 2))

                # gate h-part first: fills TensorE until the atT copies land
                # (fp8 DoubleRow, Wg pre-scaled; undone in the tanh scale)
                if g_ps is None:
                    g_ps = kvps.tile([128, HID], F32, tag="kv")
                    emit_gate_h(hT_sb, g_ps)

                # jh-major so each 512-wide half finishes as one psum group
                # and can copy out while the other half accumulates
                mo_ps = bigps.tile([128, HID], F32, tag="big")
                mo_sb = csb.tile([128, HID], BF16, tag="mo")
                for jh in range(NJH):
                    sl = slice(jh * 512, (jh + 1) * 512)
                    for ic in range(NIC):
                        nc.tensor.matmul(
                            mo_ps[:, sl],
                            atT_sb[:, ic, :],
                            wo_sb[:, ic, sl],
                            start=(ic == 0), stop=(ic == NIC - 1),
                        )
                    nc.scalar.copy(mo_sb[:, sl], mo_ps[:, sl])

                # gate mo-part via the host-fused Wf = Wo @ Wg2, straight
                # from the transposed attnout (no mo transpose needed)
                for j in range(NIC // 2):
                    for jh in range(NJH):
                        sl = slice(jh * 512, (jh + 1) * 512)
                        nc.tensor.matmul(
                            g_ps[:, sl], atT8_sb[:, 2 * j:2 * j + 2, :],
                            wf_sb[:, 2 * j:2 * j + 2, sl],
                            start=False, stop=(j == NIC // 2 - 1), perf_mode=DR)

                # epilogue in 512-wide halves: the serial chain
                # (add->tanh->mult->add->square) pipelines across DVE/ScalarE
                # so the final tile's exposed tail is ~half as long; bf16
                # intermediates keep the DVE ops in 2x packed mode
                gb_sb = csb.tile([128, HID], BF16, tag="gb")
                u_sb = csb.tile([128, HID], BF16, tag="u")
                v_sb = csb.tile([128, HID], BF16, tag="v")
                yo_sb = csb.tile([128, HID], F32, tag="yo")
                acc2 = stp.tile([128, 2, 2], F32, tag="acc2")
                # u = h + 0.5*mo is gate-independent; runs as soon as mo lands
                nc.vector.scalar_tensor_tensor(
                    u_sb[:], mo_sb[:], 0.5, h_sb[:], op0=OP.mult, op1=OP.add)
                for c in range(2):
                    sl = slice(c * 512, (c + 1) * 512)
                    # g_ps holds WS*gate_in (fp8 weights pre-scaled); bgB is
                    # pre-scaled by WS; the tanh scale folds in 1/WS;
                    # sigmoid(x) = 0.5*tanh(x/2) + 0.5
                    nc.vector.tensor_add(gb_sb[:, sl], g_ps[:, sl], bgb_sb[:, sl])
                    nc.scalar.activation(gb_sb[:, sl], gb_sb[:, sl], AF.Tanh,
                                         scale=0.5 / (WS * WS))
                    # aug = h + g*mo = (h + 0.5*mo) + (0.5*mo)*tanh
                    nc.vector.scalar_tensor_tensor(
                        v_sb[:, sl], gb_sb[:, sl], 0.5, mo_sb[:, sl],
                        op0=OP.mult, op1=OP.mult)
                    nc.vector.scalar_tensor_tensor(
                        u_sb[:, sl], u_sb[:, sl], 0.0, v_sb[:, sl],
                        op0=OP.add, op1=OP.add,
                        accum_out=acc2[:, 0, c:c + 1])
                    # square's tensor output is scrap (gb is dead after v)
                    nc.scalar.activation(
                        gb_sb[:, sl], u_sb[:, sl], AF.Square,
                        accum_out=acc2[:, 1, c:c + 1])

                st2 = stp.tile([128, 2], F32, tag="st2")
                nc.vector.reduce_sum(st2[:], acc2[:], axis=AX.X)
                # ---- LayerNorm finalize, per tile, VectorE only ----
                mean = stp.tile([128, 1], F32, tag="mean")
                nc.vector.tensor_scalar_mul(mean[:], st2[:, 0:1], 1.0 / HID)
                m2 = stp.tile([128, 1], F32, tag="m2")
                nc.vector.tensor_mul(m2[:], mean[:], mean[:])
                nc.vector.tensor_scalar_add(m2[:], m2[:], -LN_EPS)
                vpe = stp.tile([128, 1], F32, tag="vpe")
                nc.vector.scalar_tensor_tensor(
                    vpe[:], st2[:, 1:2], 1.0 / HID, m2[:],
                    op0=OP.mult, op1=OP.subtract)
                # rstd = 1/sqrt(vpe): quake init + 2 Newton iterations
                y = stp.tile([128, 1], F32, tag="y")
                yi = y[:].bitcast(I32)
                nc.vector.tensor_scalar(
                    yi, vpe[:].bitcast(I32), 1, None,
                    op0=OP.logical_shift_right)
                nc.vector.tensor_scalar(
                    yi, yi, -RSQRT_MAGIC, -1,
                    op0=OP.add, op1=OP.mult)
                yy = stp.tile([128, 1], F32, tag="yy")
                hw = stp.tile([128, 1], F32, tag="hw")
                for _ in range(2):
                    nc.vector.tensor_mul(yy[:], y[:], y[:])
                    nc.vector.tensor_mul(yy[:], yy[:], vpe[:])
                    nc.vector.tensor_scalar(
                        hw[:], yy[:], -0.5, 1.5, op0=OP.mult, op1=OP.add)
                    nc.vector.tensor_mul(y[:], y[:], hw[:])

                # yout = (aug - mean)*rstd*lng + lnb, halves so the first
                # out-DMA leaves while the second half computes
                for c in range(2):
                    sl = slice(c * 512, (c + 1) * 512)
                    # per-partition scalar pointers (mean/rstd) are DVE-only
                    nc.vector.scalar_tensor_tensor(
                        v_sb[:, sl], u_sb[:, sl], mean[:], lng_sb[:, sl],
                        op0=OP.subtract, op1=OP.mult)
                    nc.vector.scalar_tensor_tensor(
                        yo_sb[:, sl], v_sb[:, sl], y[:], lnb_sb[:, sl],
                        op0=OP.mult, op1=OP.add)
                    nc.sync.dma_start(out_d.ap()[t, :, sl], yo_sb[:, sl])

            emit_A(0)
            for t in range(nt):
                emit_B(t)
                if t + 2 < nt:
                    load_hT(t + 2)
                if t + 1 < nt:
                    emit_A(t + 1)
                if t == 0:
                    # everything below is first consumed in C(0), a full
                    # iteration later, so it is kept out of the startup
                    # window (the first ~15us are DMA-bandwidth-bound).
                    # The gpsimd engine would otherwise fire the Wo/Wg
                    # triggers immediately; a 1-element copy that depends on
                    # B(0)'s softmax holds them back until the startup
                    # streams have drained.
                    gdep = stp.tile([1, 1], F32, tag="gdep")
                    nc.gpsimd.tensor_copy(gdep[:], e_ref[0][:1, 0, :1])
                    load_w(wo_sb, wo_d, NIC, step=4, q=nc.gpsimd)
                    load_w(wg_sb, wg_d, NIC, step=4, q=nc.gpsimd)
                    load_w(wf_sb, wf_d, NIC, step=4, q=nc.gpsimd)
                    emit_cstream(0)
                else:
                    if t == 1:
                        # LN constants are first consumed in C(0)'s epilogue;
                        # emitting them here keeps their 1.5MB out of both
                        # the startup window and B(1)'s kv stream
                        bgb_sb = eyep.tile([128, HID], F32, tag="bgb")
                        nc.sync.dma_start(bgb_sb[:], bgb_d.ap())
                        lng_sb = eyep.tile([128, HID], F32, tag="lng")
                        nc.sync.dma_start(lng_sb[:], lng_d.ap())
                        lnb_sb = eyep.tile([128, HID], F32, tag="lnb")
                        nc.sync.dma_start(lnb_sb[:], lnb_d.ap())
                    emit_C(t - 1)
            # last tile: the gate h-part borrows the kv psum tag so it can
            # run during the final softmax/normalize drain, and dependency-
            # free transposes keep the PE at full pstate until the aoT
            # transpose is ready (a cold pstate otherwise doubles the final
            # C stage's matmul times)
            gl_ps = kvps.tile([128, HID], F32, tag="kv")
            emit_gate_h(cstream[nt - 1][0], gl_ps)
            warm_pe(60, bigps)
            emit_C(nt - 1, g_ps=gl_ps)

        hp_cm.__exit__(None, None, None)    # release hT stream
        pWKV_cm.__exit__(None, None, None)  # release Wq/Wk/Wv
        eyep_cm.__exit__(None, None, None)  # release consts
        pAB_cm.__exit__(None, None, None)   # release q stream
        pAO_cm.__exit__(None, None, None)   # release attnout
        pWO_cm.__exit__(None, None, None)   # release Wo/Wg

    nc.compile()
    return nc


def _prep_core(hs, mk, mv, nt):
    """Host-side lossless layout prep for one core's shard."""
    hT = np.ascontiguousarray(
        hs.reshape(nt, 128, NIC, 128).transpose(0, 3, 2, 1))      # [t,p,ic,b]
    h = np.ascontiguousarray(hs.reshape(nt, 128, HID))
    mkT = np.ascontiguousarray(
        mk.reshape(nt, 128, TOPK, NIC, 128).transpose(0, 2, 4, 3, 1))
    mvT = np.ascontiguousarray(
        mv.reshape(nt, 128, TOPK, NIC, 128).transpose(0, 2, 4, 3, 1))
    return hT, h, mkT, mvT


def kernel(**inputs):
    hs = np.asarray(inputs["hidden_state"], dtype=np.float32)
    mk = np.asarray(inputs["memory_keys"], dtype=np.float32)
    mv = np.asarray(inputs["memory_values"], dtype=np.float32)

    import ml_dtypes
    bf = ml_dtypes.bfloat16
    f8 = ml_dtypes.float8_e4m3
    wq = np.ascontiguousarray(
        np.asarray(inputs["Wq"], np.float32).reshape(NIC, 128, HID).transpose(1, 0, 2)).astype(bf)
    wk_t = np.ascontiguousarray(
        np.asarray(inputs["Wk"], np.float32).reshape(NIC, 128, HID).transpose(1, 0, 2))
    wk = (wk_t * WS).astype(f8)
    wkb = wk_t.astype(bf)
    wv = np.ascontiguousarray(
        np.asarray(inputs["Wv"], np.float32).reshape(NIC, 128, HID).transpose(1, 0, 2) * WS).astype(f8)
    wo_f = np.asarray(inputs["Wo"], np.float32)
    wg_f = np.asarray(inputs["Wg"], np.float32)
    wo = np.ascontiguousarray(
        wo_f.reshape(NIC, 128, HID).transpose(1, 0, 2) * (1.0 / WS)).astype(bf)
    wg = np.ascontiguousarray(
        wg_f[:HID].reshape(NIC, 128, HID).transpose(1, 0, 2) * WS).astype(f8)
    wf = np.ascontiguousarray(
        (wo_f @ wg_f[HID:]).reshape(NIC, 128, HID).transpose(1, 0, 2) * WS).astype(f8)
    bgb = np.ascontiguousarray(
        np.broadcast_to(np.asarray(inputs["bg"], np.float32) * (WS * WS), (128, HID)))
    lng = np.ascontiguousarray(
        np.broadcast_to(np.asarray(inputs["ln_g"], np.float32), (128, HID)))
    lnb = np.ascontiguousarray(
        np.broadcast_to(np.asarray(inputs["ln_b"], np.float32), (128, HID)))
    eyeb = np.eye(128, dtype=bf)

    if "nc" not in _CACHE:
        _CACHE["nc"] = _build(NT)
    nc = _CACHE["nc"]

    in_maps = []
    for c in range(N_CORES):
        sl = slice(c * BC, (c + 1) * BC)
        hT, h, mkT, mvT = _prep_core(hs[sl], mk[sl], mv[sl], NT)
        in_maps.append({
            "hTb": hT.astype(bf), "hT8": (hT * WS).astype(f8),
            "hB": h.astype(bf),
            "mkT": mkT[:, :TOPK - 1].astype(f8),
            "mkTb": np.ascontiguousarray(mkT[:, TOPK - 1]).astype(bf),
            "mvT": mvT.astype(f8),
            "Wq": wq, "Wk": wk, "WkB": wkb, "Wv": wv, "Wo": wo, "Wg": wg,
            "Wf": wf, "bgB": bgb, "eyeB": eyeb, "lngB": lng, "lnbB": lnb,
        })

    res = run_bass_kernel_spmd(nc, in_maps, core_ids=list(range(N_CORES)),
                               trace=TRACE)
    kernel.last_result = res
    out = np.concatenate(
        [r["out"].reshape(BC, HID) for r in res.results], axis=0)
    return out


kernel.last_result = None
